# revision 44
# baseline (speedup 1.0000x reference)
"""Cumulative (causal) LayerNorm Trainium2 Bass kernel.

Problem: inputs [B=8, K=8000, H=512] f32, gamma/beta [1, 512].
At step k, normalize frame k by mean/var computed over the prefix
inputs[:, :k+1, :] (time and feature dims), then scale/shift by gamma/beta.

Sharding: data-parallel over batch B across the 8 NeuronCores (one sample
per core), gamma/beta/constants replicated. No cross-core communication.

Per-core layout (segmented): K = NSEG * P * FS frames; frame
    k = s*(P*FS) + p*FS + f      (s = segment, p = partition, f = tile-in-seg)
Global k-tile index t = s*FS + f in [0, 64).

fp16 I/O: x and out cross HBM as fp16 (halves DMA bytes; the 2e-2 rel-err
budget has ~7x margin over fp16 quantization, verified on host). All stats
accumulate in f32 on-chip.

Engine assignment (balanced against the TRN2 cost model, where ACT ops cost
(N+~372)/1.2 ns, DVE tensor_scalar runs 4x packed, tensor_tensor 2x, and
custom/reduce ops only 1x):
  stats:  most frames: ACT Square+accum -> per-frame sumsqs; DVE
          tensor_scalar+accum (4x) -> per-frame sums. First bn_per_seg
          frames of the leading bn_segs segments instead use DVE bn_stats
          (one 1x op yields count/mean/M2 for even+odd halves; 5 small ops
          reconstruct sums/sumsqs) - fills DVE's early idle while ACT waits
          for the first loads, trims total ACT work.
  scan:   native tensor_tensor_scan along each partition's FS frames, then
          one PE matmul pair: strictly-upper-triangular ustrict @ seg-totals
          (exclusive cross-partition carry) accumulated with row124-ones @
          previous segment's final cum column (scalar carry broadcast).
  apply:  per-frame DVE tensor_scalar affine (4x): x <- x*invstd - mean*invstd,
          then ONE batched tensor_tensor gamma multiply per segment (2x)
          through a stride-0 broadcast view of gamma. Leading segments'
          gamma runs on the otherwise-idle Pool engine; the last segment
          uses per-frame gamma so its stores drain incrementally.
  dma:    input chunks on the SP HW-DGE ring; stores paired into 2MB SWDGE
          transfers (one fixed cost per pair), last segment split in halves
          for tail drain; setup DMAs (gamma + concatenated consts) go on the
          SWDGE path so the first input chunk starts immediately.
Segment s+1's loads overlap segment s's stores -> no global barrier.
"""

import numpy as np

import concourse.bass as bass
import concourse.tile as tile
from concourse import bacc, mybir
from concourse import bass_utils

B, K, H = 8, 8000, 512
P = 125           # partitions
NSEG = 8          # segments per sample
FS = 8            # k-tiles per segment  (K = NSEG * P * FS)
F = NSEG * FS     # 64 k-tiles total
EPS = 1e-8
N_CORES = 8

FP32 = mybir.dt.float32
FP16 = mybir.dt.float16


def _global_k(p, t, fs=FS):
    # frame index of (partition p, global tile t)
    s, f = t // fs, t % fs
    return s * (P * fs) + p * fs + f


def _make_consts(nseg: int = NSEG):
    # consts[:, 0:F] = -1/count, consts[:, F:2F] = +1/count, count = (k+1)*H
    # consts[:, 2F:2F+P] = ustrict, consts[:, 2F+P:2F+2P] = row124.
    # One concatenated tensor -> one setup DMA instead of three.
    pp, tt = np.meshgrid(np.arange(P), np.arange(F), indexing="ij")
    k = _global_k(pp, tt, F // nseg).astype(np.float64)
    inv_cnt = 1.0 / ((k + 1) * H)
    invc2 = np.concatenate([-inv_cnt, inv_cnt], axis=1).astype(np.float32)
    # ustrict[q, i] = 1 if q < i  (exclusive prefix over partitions via matmul)
    ustrict = np.triu(np.ones((P, P), dtype=np.float32), k=1)
    # row124[q, i] = 1 if q == P-1: broadcasts the previous segment's final
    # cum value (held by partition P-1) to every partition via matmul.
    row124 = np.zeros((P, P), dtype=np.float32)
    row124[P - 1, :] = 1.0
    return np.ascontiguousarray(
        np.concatenate([invc2, ustrict, row124], axis=1))


def _build_kernel(with_beta: bool, reps: int = 1,
                  rep_barrier: bool = False, nseg: int = NSEG,
                  apply_mode: str = "ts_ttb",
                  sum_mode: str = "dve_ts",
                  sum_act_frames: int = 0,
                  sq_dve_frames: int = 0,
                  hw_loop: int = 0,
                  load_mode: str = "sync",
                  store_mode: str = "pair",
                  setup_on: str = "gpsimd",
                  bn_segs: int = 3,
                  bn_pos: str = "lead",
                  bn_per_seg: int = 5,
                  pool_gamma_segs: int = 3,
                  last_ttmul: bool = True,
                  ttmul_tail: int = 3,
                  split_first_load: bool = True,
                  last_act_affines: int = 4,
                  last_store_quarters: bool = False,
                  touch: bool = True,
                  parts: int = 63) -> bass.Bass:
    # parts bitmask (timing microbenchmarks only; full kernel = 63):
    #   1=LOAD 2=SUMS 4=SQUARES 8=SCAN+STATS 16=APPLY 32=STORE
    P_LOAD, P_SUMS, P_SQ, P_SCAN, P_APPLY, P_STORE = 1, 2, 4, 8, 16, 32
    # reps>1 repeats the whole pipeline (timing harness only): wall-clock
    # difference between reps=R and reps=1 isolates per-iteration HW time
    # from PJRT/axon dispatch overhead. rep_barrier approximates single-shot
    # latency by separating reps with an all-engine barrier.
    #
    # apply_mode: "amr" = fused affine_mul_reduce per frame (1x custom DVE);
    #             "ts_tt" = tensor_scalar affine (4x) + tensor_mul gamma (2x).
    # sum_act_frames: how many of each segment's FS frame-sums go to ACT
    #             (Identity+accum) instead of DVE tensor_scalar+accum.
    #
    # Bacc (not raw Bass): its compile() step legalizes instructions with
    # multiple sync waits into EventSemaphore instructions; the TRN2 engine
    # instruction encodings only fit one wait each.
    nc = bacc.Bacc("TRN2", target_bir_lowering=False, debug=False,
                   num_devices=N_CORES)
    x_d = nc.dram_tensor("x", [K, H], FP16, kind="ExternalInput").ap()
    gamma_d = nc.dram_tensor("gamma", [1, H], FP16, kind="ExternalInput").ap()
    beta_d = nc.dram_tensor("beta", [1, H], FP16, kind="ExternalInput").ap()
    consts_d = nc.dram_tensor("consts", [P, 2 * F + 2 * P], FP32,
                              kind="ExternalInput").ap()
    out_d = nc.dram_tensor("out", [K, H], FP16, kind="ExternalOutput").ap()

    # [NSEG, P, FS, H]: frame k = s*(P*FS) + p*FS + f
    fs = F // nseg
    x_v = x_d.rearrange("(s p f) h -> s p f h", p=P, f=fs)
    out_v = out_d.rearrange("(s p f) h -> s p f h", p=P, f=fs)
    # partition-first view for paired (2-segment) stores
    out_pv = out_d.rearrange("(s p f) h -> p s f h", p=P, f=fs)

    with tile.TileContext(nc) as tc:
        with (
            tc.tile_pool(name="xbuf", bufs=1) as xpool,
            tc.tile_pool(name="small", bufs=1) as small,
            tc.tile_pool(name="psum", bufs=2, space="PSUM") as psum,
        ):
            X = xpool.tile([P, F, H], FP16)   # X[:, t, :], t = s*FS + f

            # setup DMAs go on the SWDGE (store) path so the input HWDGE
            # rings start streaming x chunks immediately (single-shot lead-in)
            setup_eng = nc.gpsimd if setup_on == "gpsimd" else nc.sync

            G = small.tile([P, H], FP16)
            setup_eng.dma_start(G[:, :], gamma_d.to_broadcast((P, H)))
            # batched per-segment gamma multiply reads G through a
            # stride-0 broadcast view -> no replicated tile, no extra DMAs
            G8v = G[:, :].unsqueeze(1).broadcast_to((P, fs, H))
            Bt = None
            if with_beta:
                Bt = small.tile([P, H], FP16, tag="beta")
                setup_eng.dma_start(Bt[:, :], beta_d.to_broadcast((P, H)))
            consts = small.tile([P, 2 * F + 2 * P], FP32, tag="consts")
            setup_eng.dma_start(consts[:, :], consts_d)
            invc2 = consts[:, 0:2 * F]
            ustrict = consts[:, 2 * F:2 * F + P]
            row124 = consts[:, 2 * F + P:2 * F + 2 * P]
            eps_t = small.tile([P, 1], FP32, tag="eps")
            nc.vector.memset(eps_t[:, :], EPS)
            # zb must be produced on ACT: the Square+accum activation below
            # carries a same-engine accumulator wait, and its single encodable
            # sync wait must go to the X-chunk DMA.
            zb = small.tile([P, 1], FP32, tag="zb")
            nc.scalar.memzero(zb[:, :])

            S = small.tile([P, 2 * F], FP32, tag="S")    # sums | sumsqs
            C = small.tile([P, 2 * F], FP32, tag="C")    # global cums
            M = small.tile([P, 2 * F], FP32, tag="M")    # [-mean | E[x^2]]
            Msq = small.tile([P, F], FP32, tag="Msq")
            V = small.tile([P, F], FP32, tag="V")
            ISD = small.tile([P, F], FP32, tag="ISD")
            NMB = small.tile([P, F], FP32, tag="NMB")
            carryS = small.tile([P, 2 * nseg], FP32, tag="carryS")
            sqscr = small.tile([P, H], FP16, tag="sqscr")    # ACT square out
            sumscr = small.tile([P, H], FP16, tag="sumscr")  # DVE TS-sum out
            sqscr2 = None
            if sq_dve_frames > 0:
                # separate scratch for DVE-side squares: avoids false WAW
                # deps between ACT and DVE square instructions
                sqscr2 = small.tile([P, H], FP16, tag="sqscr2")
            BNS = BNM = BNA = None
            if bn_segs > 0:
                # bn_stats path (trailing segments): one DVE op per frame
                # yields count/mean/M2 for even+odd element halves; sums and
                # sumsqs reconstruct with 6 small per-segment ops.
                BNS = small.tile([P, fs, 6], FP32, tag="BNS")
                BNM = small.tile([P, fs, 2], FP32, tag="BNM")   # m^2 pairs
                BNA = small.tile([P, fs, 2], FP32, tag="BNA")   # v + m^2
                bntmp = small.tile([P, 2 * fs], FP32, tag="bntmp")
            amracc = small.tile([P, 1], FP32, tag="amracc")  # discarded accum

            if not (parts & P_SCAN):
                # microbench modes that skip stats still apply/scan-read
                # these tiles; give them defined values once
                nc.vector.memset(ISD[:, :], 1.0)
                nc.vector.memset(NMB[:, :], 0.0)
                nc.vector.memset(S[:, :], 0.0)
            elif not (parts & P_SUMS) or not (parts & P_SQ):
                # scan enabled but one of its producers disabled: define S
                nc.vector.memset(S[:, :], 0.0)

            carryP = psum.tile([P, 2], FP32)
            pe_touch = psum.tile([1, 1], FP32, tag="pe_touch")

            # touchers: one tiny engine-read per DMA so later ops on that
            # engine (whose encodings fit one sync wait, already used by
            # their same-engine chains) never need to also wait on a DMA sem.
            touch_t = small.tile([1, nseg], FP16, tag="touch")
            touchv = small.tile([1, nseg], FP16, tag="touchv")

            # strided views pairing the sum and sumsq halves: [P, 2, F]
            Cr = C[:, :].rearrange("p (a b) -> p a b", b=F)
            Mr = M[:, :].rearrange("p (a b) -> p a b", b=F)
            Ir = invc2[:, :].rearrange("p (a b) -> p a b", b=F)

            # absorb the ustrict/row124 DMA wait on PE once
            nc.tensor.matmul(pe_touch[0:1, 0:1], row124[0:1, 0:1],
                             ustrict[0:1, 0:1], start=True, stop=True)

            def _rep_body():
              for s in range(nseg):
                t0 = s * fs
                # ---- load + per-frame sum/sumsq for this segment ---------
                # loads alternate between the two HWDGE rings (SP / ACT) so
                # per-DMA completion latencies overlap across rings
                if parts & P_LOAD:
                    load_eng = (nc.scalar if load_mode == "split" and s % 2
                                else nc.sync)
                    if s == 0 and split_first_load:
                        # halve the first load so compute starts ~1.3us
                        # earlier (everything downstream shifts left)
                        h2 = fs // 2
                        load_eng.dma_start(X[:, t0:t0 + h2, :],
                                           x_v[s][:, 0:h2, :])
                        load_eng.dma_start(X[:, t0 + h2:t0 + fs, :],
                                           x_v[s][:, h2:fs, :])
                    else:
                        load_eng.dma_start(X[:, t0:t0 + fs, :], x_v[s])
                    if touch:
                        nc.scalar.copy(touch_t[0:1, s:s + 1], X[0:1, t0, 0:1])
                        nc.vector.tensor_scalar(
                            out=touchv[0:1, s:s + 1], in0=X[0:1, t0, 0:1],
                            scalar1=1.0, scalar2=None,
                            op0=mybir.AluOpType.mult)
                bn_seg = (s < bn_segs if bn_pos == "lead"
                          else s >= nseg - bn_segs)
                # first `bn_j` frames of a bn segment take the DVE bn_stats
                # path; the rest stay on ACT Square so both engines start
                # chewing the segment immediately
                bn_j = (bn_per_seg or fs) if bn_seg else 0
                if bn_j and (parts & P_SQ) and (parts & P_SUMS):
                    for i in range(bn_j):
                        nc.vector.bn_stats(BNS[:, i, :], X[:, t0 + i, :])
                    # sums: S[f] = 256*(m_even + m_odd)
                    nc.vector.tensor_add(bntmp[:, 0:bn_j],
                                         BNS[:, 0:bn_j, 1], BNS[:, 0:bn_j, 4])
                    nc.vector.tensor_scalar_mul(S[:, t0:t0 + bn_j],
                                                bntmp[:, 0:bn_j], 256.0)
                    # sumsqs: BNS[...,2|5] is count*var, so
                    # S[F+f] = (cv_e + cv_o) + 256*(m_e^2 + m_o^2)
                    mv = BNS[:, :, :].rearrange("p f (u c) -> p f u c", u=2)
                    nc.vector.tensor_mul(BNM[:, 0:bn_j, :],
                                         mv[:, 0:bn_j, :, 1],
                                         mv[:, 0:bn_j, :, 1])  # m^2 e|o
                    nc.vector.scalar_tensor_tensor(
                        out=BNA[:, 0:bn_j, :], in0=BNM[:, 0:bn_j, :],
                        scalar=256.0, in1=mv[:, 0:bn_j, :, 2],
                        op0=mybir.AluOpType.mult,
                        op1=mybir.AluOpType.add)   # 256*m^2 + count*var
                    nc.vector.tensor_add(S[:, F + t0:F + t0 + bn_j],
                                         BNA[:, 0:bn_j, 0], BNA[:, 0:bn_j, 1])
                for i, f in enumerate(range(t0, t0 + fs)):
                    if i < bn_j and (parts & P_SQ) and (parts & P_SUMS):
                        continue
                    if not (parts & P_SQ):
                        pass
                    elif i < sq_dve_frames:
                        # sumsq on DVE: x*x tensor_tensor_reduce (2x packed)
                        nc.vector.tensor_tensor_reduce(
                            out=sqscr2[:, :], in0=X[:, f, :], in1=X[:, f, :],
                            scale=1.0, scalar=0.0,
                            op0=mybir.AluOpType.mult,
                            op1=mybir.AluOpType.add,
                            accum_out=S[:, F + f:F + f + 1],
                        )
                    else:
                        nc.scalar.activation(
                            out=sqscr[:, :], in_=X[:, f, :],
                            func=mybir.ActivationFunctionType.Square,
                            bias=zb[:, :], scale=1.0,
                            accum_out=S[:, F + f:F + f + 1],
                        )
                    if not (parts & P_SUMS):
                        pass
                    elif i < sum_act_frames:
                        # frame-sum on ACT: Identity+accum
                        nc.scalar.activation(
                            out=sumscr[:, :], in_=X[:, f, :],
                            func=mybir.ActivationFunctionType.Identity,
                            bias=zb[:, :], scale=1.0,
                            accum_out=S[:, f:f + 1],
                        )
                    elif sum_mode == "dve_ts":
                        # frame-sum on DVE: copy+accum at 4x packed mode
                        nc.vector.tensor_scalar(
                            out=sumscr[:, :], in0=X[:, f, :],
                            scalar1=1.0, scalar2=0.0,
                            op0=mybir.AluOpType.mult,
                            op1=mybir.AluOpType.add,
                            accum_out=S[:, f:f + 1],
                        )
                if (parts & P_SUMS) and sum_mode == "reduce":
                    # batched DVE reduce over the segment (1x mode)
                    nc.vector.reduce_sum(S[:, t0:t0 + fs],
                                         X[:, t0:t0 + fs, :],
                                         axis=mybir.AxisListType.X)

                if parts & P_SCAN:
                    # ---- causal scan for this segment --------------------
                    # intra-partition inclusive prefix over the FS frames
                    # each partition owns (fp32 recurrence on DVE)
                    nc.vector.tensor_tensor_scan(
                        out=C[:, t0:t0 + fs], data0=S[:, t0:t0 + fs],
                        data1=S[:, t0:t0 + fs], initial=0.0,
                        op0=mybir.AluOpType.add, op1=mybir.AluOpType.bypass)
                    nc.vector.tensor_tensor_scan(
                        out=C[:, F + t0:F + t0 + fs],
                        data0=S[:, F + t0:F + t0 + fs],
                        data1=S[:, F + t0:F + t0 + fs], initial=0.0,
                        op0=mybir.AluOpType.add, op1=mybir.AluOpType.bypass)
                    # cross-partition exclusive carry (+ prev segment total):
                    # carry[p] = sum_{q<p} seg_total[q] + prev_seg_final
                    totals = Cr[:, :, t0 + fs - 1]          # [P, 2] strided
                    nc.tensor.matmul(carryP[:, 0:2], ustrict[:, :], totals,
                                     start=True, stop=(s == 0))
                    if s > 0:
                        prevfinal = Cr[:, :, t0 - 1]        # already global
                        nc.tensor.matmul(carryP[:, 0:2], row124[:, :],
                                         prevfinal, start=False, stop=True)
                    cS = carryS[:, 2 * s:2 * s + 2]
                    nc.scalar.copy(cS[:, :], carryP[:, :])
                    nc.vector.tensor_scalar_add(C[:, t0:t0 + fs],
                                                C[:, t0:t0 + fs], cS[:, 0:1])
                    nc.vector.tensor_scalar_add(C[:, F + t0:F + t0 + fs],
                                                C[:, F + t0:F + t0 + fs],
                                                cS[:, 1:2])

                    # ---- stats for this segment --------------------------
                    # M = C * invc2: [-mean | E[x^2]] (both halves, 3D AP).
                    # C must stay intact: the next segment's carry matmul
                    # reads this segment's final cum column.
                    nc.vector.tensor_mul(Mr[:, :, t0:t0 + fs],
                                         Cr[:, :, t0:t0 + fs],
                                         Ir[:, :, t0:t0 + fs])
                    nc.vector.tensor_mul(Msq[:, t0:t0 + fs], M[:, t0:t0 + fs],
                                         M[:, t0:t0 + fs])          # mean^2
                    nc.vector.tensor_sub(V[:, t0:t0 + fs],
                                         M[:, F + t0:F + t0 + fs],
                                         Msq[:, t0:t0 + fs])        # var
                    nc.scalar.activation(out=V[:, t0:t0 + fs],
                                         in_=V[:, t0:t0 + fs],
                                         func=mybir.ActivationFunctionType.Sqrt,
                                         bias=eps_t[:, :], scale=1.0)
                    nc.vector.reciprocal(ISD[:, t0:t0 + fs], V[:, t0:t0 + fs])
                    if apply_mode == "amr" or (last_act_affines
                                               and s == nseg - 1):
                        # -mean*invstd, only where a mult-then-add form
                        # needs it (ACT affine / amr); the DVE TS affine
                        # uses add-then-mult with M directly
                        nc.vector.tensor_mul(NMB[:, t0:t0 + fs],
                                             M[:, t0:t0 + fs],
                                             ISD[:, t0:t0 + fs])

                # ---- apply + store for this segment ----------------------
                if parts & P_APPLY:
                    seg_ttmul = last_ttmul and s >= nseg - ttmul_tail
                    for f in range(t0, t0 + fs):
                        if apply_mode == "amr":
                            # out = (x*invstd + (-mean*invstd)) * gamma,
                            # fused, one rounding to fp16 at the output.
                            nc.vector.affine_mul_reduce(
                                out=X[:, f, :], accum_out=amracc[:, 0:1],
                                in0=X[:, f, :], in1=G[:, :],
                                scale=ISD[:, f:f + 1], bias=NMB[:, f:f + 1])
                        else:
                            if (s == nseg - 1 and last_act_affines
                                    and f - t0 < last_act_affines):
                                # tail: ACT is idle after its last squares,
                                # so it takes the first affines of the final
                                # segment while DVE handles rest + gammas
                                nc.scalar.activation(
                                    out=X[:, f, :], in_=X[:, f, :],
                                    func=mybir.ActivationFunctionType.Identity,
                                    bias=NMB[:, f:f + 1],
                                    scale=ISD[:, f:f + 1])
                            else:
                                # x <- (x + (-mean)) * invstd, 4x packed TS;
                                # add-then-mult uses M directly (no NMB op)
                                nc.vector.tensor_scalar(
                                    out=X[:, f, :], in0=X[:, f, :],
                                    scalar1=M[:, f:f + 1],
                                    scalar2=ISD[:, f:f + 1],
                                    op0=mybir.AluOpType.add,
                                    op1=mybir.AluOpType.mult)
                            if apply_mode == "ts_tt" or seg_ttmul:
                                # per-frame gamma: lets the final stores
                                # drain while later frames still apply
                                nc.vector.tensor_mul(X[:, f, :], X[:, f, :],
                                                     G[:, :])
                        if Bt is not None:
                            nc.vector.tensor_add(X[:, f, :], X[:, f, :],
                                                 Bt[:, :])
                    if apply_mode == "ts_ttb" and not seg_ttmul:
                        # one batched gamma multiply for the segment (2x);
                        # leading segments can offload it to the idle Pool
                        geng = (nc.gpsimd if s < pool_gamma_segs
                                else nc.vector)
                        geng.tensor_mul(X[:, t0:t0 + fs, :],
                                        X[:, t0:t0 + fs, :], G8v)
                if parts & P_STORE:
                    tail_start = nseg - (ttmul_tail if last_ttmul else 1)
                    if store_mode != "pair" or nseg < 4:
                        nc.gpsimd.dma_start(out_v[s], X[:, t0:t0 + fs, :])
                    elif s >= tail_start:
                        # tail segments apply gamma per-frame, so stores
                        # chunk finer and stream as frames complete
                        nq = 4 if s == nseg - 1 else 2
                        qq = fs // nq
                        for q in range(nq):
                            nc.gpsimd.dma_start(
                                out_v[s][:, q * qq:(q + 1) * qq],
                                X[:, t0 + q * qq:t0 + (q + 1) * qq, :])
                    elif s % 2 == 1:
                        # paired 2MB store of segments s-1, s: one SWDGE
                        # fixed cost amortized over both
                        src = X[:, t0 - fs:t0 + fs, :].rearrange(
                            "p (u f) h -> p u f h", u=2)
                        nc.gpsimd.dma_start(out_pv[:, s - 1:s + 1], src)
                    elif s == tail_start - 1 and tail_start % 2 == 1:
                        # odd leftover before the tail: single store
                        nc.gpsimd.dma_start(out_v[s], X[:, t0:t0 + fs, :])

            if hw_loop > 0:
                # hardware loop: hw_loop iterations of `reps` unrolled
                # pipelines each — total reps*hw_loop, small NEFF. Used by
                # the timing harness for a noise-robust on-device signal.
                # staggered_reset avoids the all-engine drain+barrier at the
                # back edge so iterations pipeline like unrolled reps do.
                with tc.For_i(0, hw_loop, staggered_reset=True):
                    for _rep in range(reps):
                        _rep_body()
            else:
                for _rep in range(reps):
                    if rep_barrier and _rep > 0:
                        tc.strict_bb_all_engine_barrier()
                    _rep_body()

    # Runs Bacc's compile passes (register allocation, EventSemaphore
    # legalization of multi-wait instructions, nop fusion).
    nc.finalize()
    return nc


_NC_CACHE: dict = {}


def kernel(**inputs: np.ndarray) -> np.ndarray:
    x = np.asarray(inputs["inputs"])
    gamma = np.asarray(inputs["gamma"], dtype=np.float32)
    beta = np.asarray(inputs["beta"], dtype=np.float32)
    assert x.shape == (B, K, H), x.shape

    x16 = np.ascontiguousarray(x.astype(np.float16))
    gamma16 = np.ascontiguousarray(gamma.reshape(1, H).astype(np.float16))
    beta16 = np.ascontiguousarray(beta.reshape(1, H).astype(np.float16))

    with_beta = bool(np.any(beta != 0.0))
    key = (with_beta, 1)
    if key not in _NC_CACHE:
        _NC_CACHE[key] = _build_kernel(with_beta, reps=1)
    nc = _NC_CACHE[key]

    consts = _make_consts()
    in_maps = [
        {
            "x": np.ascontiguousarray(x16[b]),
            "gamma": gamma16,
            "beta": beta16,
            "consts": consts,
        }
        for b in range(B)
    ]
    res = bass_utils.run_bass_kernel_spmd(nc, in_maps, core_ids=list(range(N_CORES)))
    out = np.stack([res.results[b]["out"] for b in range(B)], axis=0)
    return out.astype(np.float32)



# revision 47
# speedup vs baseline: 1.0104x; 1.0104x over previous
"""Cumulative (causal) LayerNorm Trainium2 Bass kernel.

Problem: inputs [B=8, K=8000, H=512] f32, gamma/beta [1, 512].
At step k, normalize frame k by mean/var computed over the prefix
inputs[:, :k+1, :] (time and feature dims), then scale/shift by gamma/beta.

Sharding: data-parallel over batch B across the 8 NeuronCores (one sample
per core), gamma/beta/constants replicated. No cross-core communication.

Per-core layout (segmented): K = NSEG * P * FS frames; frame
    k = s*(P*FS) + p*FS + f      (s = segment, p = partition, f = tile-in-seg)
Global k-tile index t = s*FS + f in [0, 64).

fp16 I/O: x and out cross HBM as fp16 (halves DMA bytes; the 2e-2 rel-err
budget has ~7x margin over fp16 quantization, verified on host). All stats
accumulate in f32 on-chip.

Engine assignment (balanced against the TRN2 cost model, where ACT ops cost
(N+~372)/1.2 ns, DVE tensor_scalar runs 4x packed, tensor_tensor 2x, and
custom/reduce ops only 1x):
  stats:  most frames: ACT Square+accum -> per-frame sumsqs; DVE
          tensor_scalar+accum (4x) -> per-frame sums. First bn_per_seg
          frames of the leading bn_segs segments instead use DVE bn_stats
          (one 1x op yields count/mean/M2 for even+odd halves; 5 small ops
          reconstruct sums/sumsqs) - fills DVE's early idle while ACT waits
          for the first loads, trims total ACT work.
  scan:   native tensor_tensor_scan along each partition's FS frames, then
          one PE matmul pair: strictly-upper-triangular ustrict @ seg-totals
          (exclusive cross-partition carry) accumulated with row124-ones @
          previous segment's final cum column (scalar carry broadcast).
  apply:  per-frame DVE tensor_scalar affine (4x): x <- x*invstd - mean*invstd,
          then ONE batched tensor_tensor gamma multiply per segment (2x)
          through a stride-0 broadcast view of gamma. Leading segments'
          gamma runs on the otherwise-idle Pool engine; the last segment
          uses per-frame gamma so its stores drain incrementally.
  dma:    input chunks on the SP HW-DGE ring; stores paired into 2MB SWDGE
          transfers (one fixed cost per pair), last segment split in halves
          for tail drain; setup DMAs (gamma + concatenated consts) go on the
          SWDGE path so the first input chunk starts immediately.
Segment s+1's loads overlap segment s's stores -> no global barrier.
"""

import numpy as np

import concourse.bass as bass
import concourse.tile as tile
from concourse import bacc, mybir
from concourse import bass_utils

B, K, H = 8, 8000, 512
P = 125           # partitions
NSEG = 8          # segments per sample
FS = 8            # k-tiles per segment  (K = NSEG * P * FS)
F = NSEG * FS     # 64 k-tiles total
EPS = 1e-8
N_CORES = 8

FP32 = mybir.dt.float32
FP16 = mybir.dt.float16


def _global_k(p, t, fs=FS):
    # frame index of (partition p, global tile t)
    s, f = t // fs, t % fs
    return s * (P * fs) + p * fs + f


def _make_consts(nseg: int = NSEG):
    # consts[:, 0:F] = -1/count, consts[:, F:2F] = +1/count, count = (k+1)*H
    # consts[:, 2F:2F+P] = ustrict, consts[:, 2F+P:2F+2P] = row124.
    # One concatenated tensor -> one setup DMA instead of three.
    pp, tt = np.meshgrid(np.arange(P), np.arange(F), indexing="ij")
    k = _global_k(pp, tt, F // nseg).astype(np.float64)
    inv_cnt = 1.0 / ((k + 1) * H)
    invc2 = np.concatenate([-inv_cnt, inv_cnt], axis=1).astype(np.float32)
    # ustrict[q, i] = 1 if q < i  (exclusive prefix over partitions via matmul)
    ustrict = np.triu(np.ones((P, P), dtype=np.float32), k=1)
    # row124[q, i] = 1 if q == P-1: broadcasts the previous segment's final
    # cum value (held by partition P-1) to every partition via matmul.
    row124 = np.zeros((P, P), dtype=np.float32)
    row124[P - 1, :] = 1.0
    return np.ascontiguousarray(
        np.concatenate([invc2, ustrict, row124], axis=1))


def _build_kernel(with_beta: bool, reps: int = 1,
                  rep_barrier: bool = False, nseg: int = NSEG,
                  apply_mode: str = "ts_ttb",
                  sum_mode: str = "dve_ts",
                  sum_act_frames: int = 0,
                  sq_dve_frames: int = 0,
                  hw_loop: int = 0,
                  load_mode: str = "sync",
                  store_mode: str = "pair",
                  setup_on: str = "gpsimd",
                  bn_segs: int = 3,
                  bn_pos: str = "lead",
                  bn_per_seg: int = 5,
                  pool_gamma_segs: int = 3,
                  last_ttmul: bool = True,
                  ttmul_tail: int = 3,
                  split_first_load: bool = True,
                  last_act_affines: int = 4,
                  last_store_quarters: bool = False,
                  touch: bool = True,
                  bn_on_pool: bool = True,
                  parts: int = 63) -> bass.Bass:
    # parts bitmask (timing microbenchmarks only; full kernel = 63):
    #   1=LOAD 2=SUMS 4=SQUARES 8=SCAN+STATS 16=APPLY 32=STORE
    P_LOAD, P_SUMS, P_SQ, P_SCAN, P_APPLY, P_STORE = 1, 2, 4, 8, 16, 32
    # reps>1 repeats the whole pipeline (timing harness only): wall-clock
    # difference between reps=R and reps=1 isolates per-iteration HW time
    # from PJRT/axon dispatch overhead. rep_barrier approximates single-shot
    # latency by separating reps with an all-engine barrier.
    #
    # apply_mode: "amr" = fused affine_mul_reduce per frame (1x custom DVE);
    #             "ts_tt" = tensor_scalar affine (4x) + tensor_mul gamma (2x).
    # sum_act_frames: how many of each segment's FS frame-sums go to ACT
    #             (Identity+accum) instead of DVE tensor_scalar+accum.
    #
    # Bacc (not raw Bass): its compile() step legalizes instructions with
    # multiple sync waits into EventSemaphore instructions; the TRN2 engine
    # instruction encodings only fit one wait each.
    nc = bacc.Bacc("TRN2", target_bir_lowering=False, debug=False,
                   num_devices=N_CORES)
    x_d = nc.dram_tensor("x", [K, H], FP16, kind="ExternalInput").ap()
    gamma_d = nc.dram_tensor("gamma", [1, H], FP16, kind="ExternalInput").ap()
    beta_d = nc.dram_tensor("beta", [1, H], FP16, kind="ExternalInput").ap()
    consts_d = nc.dram_tensor("consts", [P, 2 * F + 2 * P], FP32,
                              kind="ExternalInput").ap()
    out_d = nc.dram_tensor("out", [K, H], FP16, kind="ExternalOutput").ap()

    # [NSEG, P, FS, H]: frame k = s*(P*FS) + p*FS + f
    fs = F // nseg
    x_v = x_d.rearrange("(s p f) h -> s p f h", p=P, f=fs)
    out_v = out_d.rearrange("(s p f) h -> s p f h", p=P, f=fs)
    # partition-first view for paired (2-segment) stores
    out_pv = out_d.rearrange("(s p f) h -> p s f h", p=P, f=fs)

    with tile.TileContext(nc) as tc:
        with (
            tc.tile_pool(name="xbuf", bufs=1) as xpool,
            tc.tile_pool(name="small", bufs=1) as small,
            tc.tile_pool(name="psum", bufs=2, space="PSUM") as psum,
        ):
            X = xpool.tile([P, F, H], FP16)   # X[:, t, :], t = s*FS + f

            # setup DMAs go on the SWDGE (store) path so the input HWDGE
            # rings start streaming x chunks immediately (single-shot lead-in)
            setup_eng = nc.gpsimd if setup_on == "gpsimd" else nc.sync

            G = small.tile([P, H], FP16)
            setup_eng.dma_start(G[:, :], gamma_d.to_broadcast((P, H)))
            # batched per-segment gamma multiply reads G through a
            # stride-0 broadcast view -> no replicated tile, no extra DMAs
            G8v = G[:, :].unsqueeze(1).broadcast_to((P, fs, H))
            Bt = None
            if with_beta:
                Bt = small.tile([P, H], FP16, tag="beta")
                setup_eng.dma_start(Bt[:, :], beta_d.to_broadcast((P, H)))
            consts = small.tile([P, 2 * F + 2 * P], FP32, tag="consts")
            setup_eng.dma_start(consts[:, :], consts_d)
            invc2 = consts[:, 0:2 * F]
            ustrict = consts[:, 2 * F:2 * F + P]
            row124 = consts[:, 2 * F + P:2 * F + 2 * P]
            eps_t = small.tile([P, 1], FP32, tag="eps")
            nc.vector.memset(eps_t[:, :], EPS)
            # zb must be produced on ACT: the Square+accum activation below
            # carries a same-engine accumulator wait, and its single encodable
            # sync wait must go to the X-chunk DMA.
            zb = small.tile([P, 1], FP32, tag="zb")
            nc.scalar.memzero(zb[:, :])

            S = small.tile([P, 2 * F], FP32, tag="S")    # sums | sumsqs
            C = small.tile([P, 2 * F], FP32, tag="C")    # global cums
            M = small.tile([P, 2 * F], FP32, tag="M")    # [-mean | E[x^2]]
            Msq = small.tile([P, F], FP32, tag="Msq")
            V = small.tile([P, F], FP32, tag="V")
            ISD = small.tile([P, F], FP32, tag="ISD")
            NMB = small.tile([P, F], FP32, tag="NMB")
            carryS = small.tile([P, 2 * nseg], FP32, tag="carryS")
            sqscr = small.tile([P, H], FP16, tag="sqscr")    # ACT square out
            sumscr = small.tile([P, H], FP16, tag="sumscr")  # DVE TS-sum out
            sqscr2 = None
            if sq_dve_frames > 0:
                # separate scratch for DVE-side squares: avoids false WAW
                # deps between ACT and DVE square instructions
                sqscr2 = small.tile([P, H], FP16, tag="sqscr2")
            BNS = BNM = BNA = None
            if bn_segs > 0:
                # bn_stats path (trailing segments): one DVE op per frame
                # yields count/mean/M2 for even+odd element halves; sums and
                # sumsqs reconstruct with 6 small per-segment ops.
                BNS = small.tile([P, fs, 6], FP32, tag="BNS")
                BNM = small.tile([P, fs, 2], FP32, tag="BNM")   # m^2 pairs
                BNA = small.tile([P, fs, 2], FP32, tag="BNA")   # v + m^2
                bntmp = small.tile([P, 2 * fs], FP32, tag="bntmp")
            amracc = small.tile([P, 1], FP32, tag="amracc")  # discarded accum

            if not (parts & P_SCAN):
                # microbench modes that skip stats still apply/scan-read
                # these tiles; give them defined values once
                nc.vector.memset(ISD[:, :], 1.0)
                nc.vector.memset(NMB[:, :], 0.0)
                nc.vector.memset(S[:, :], 0.0)
            elif not (parts & P_SUMS) or not (parts & P_SQ):
                # scan enabled but one of its producers disabled: define S
                nc.vector.memset(S[:, :], 0.0)

            carryP = psum.tile([P, 2], FP32)
            pe_touch = psum.tile([1, 1], FP32, tag="pe_touch")

            # touchers: one tiny engine-read per DMA so later ops on that
            # engine (whose encodings fit one sync wait, already used by
            # their same-engine chains) never need to also wait on a DMA sem.
            touch_t = small.tile([1, nseg], FP16, tag="touch")
            touchv = small.tile([1, nseg], FP16, tag="touchv")

            # strided views pairing the sum and sumsq halves: [P, 2, F]
            Cr = C[:, :].rearrange("p (a b) -> p a b", b=F)
            Mr = M[:, :].rearrange("p (a b) -> p a b", b=F)
            Ir = invc2[:, :].rearrange("p (a b) -> p a b", b=F)

            # absorb the ustrict/row124 DMA wait on PE once
            nc.tensor.matmul(pe_touch[0:1, 0:1], row124[0:1, 0:1],
                             ustrict[0:1, 0:1], start=True, stop=True)

            def _rep_body():
              for s in range(nseg):
                t0 = s * fs
                # ---- load + per-frame sum/sumsq for this segment ---------
                # loads alternate between the two HWDGE rings (SP / ACT) so
                # per-DMA completion latencies overlap across rings
                if parts & P_LOAD:
                    load_eng = (nc.scalar if load_mode == "split" and s % 2
                                else nc.sync)
                    if s == 0 and split_first_load:
                        # halve the first load so compute starts ~1.3us
                        # earlier (everything downstream shifts left)
                        h2 = fs // 2
                        load_eng.dma_start(X[:, t0:t0 + h2, :],
                                           x_v[s][:, 0:h2, :])
                        load_eng.dma_start(X[:, t0 + h2:t0 + fs, :],
                                           x_v[s][:, h2:fs, :])
                    else:
                        load_eng.dma_start(X[:, t0:t0 + fs, :], x_v[s])
                    if touch:
                        nc.scalar.copy(touch_t[0:1, s:s + 1], X[0:1, t0, 0:1])
                        nc.vector.tensor_scalar(
                            out=touchv[0:1, s:s + 1], in0=X[0:1, t0, 0:1],
                            scalar1=1.0, scalar2=None,
                            op0=mybir.AluOpType.mult)
                bn_seg = (s < bn_segs if bn_pos == "lead"
                          else s >= nseg - bn_segs)
                # first `bn_j` frames of a bn segment take the DVE bn_stats
                # path; the rest stay on ACT Square so both engines start
                # chewing the segment immediately
                bn_j = (bn_per_seg or fs) if bn_seg else 0
                if bn_j and (parts & P_SQ) and (parts & P_SUMS):
                    # reconstruction smalls can run on the Pool engine,
                    # which idles until the first stores (~14us)
                    be = nc.gpsimd if bn_on_pool else nc.vector
                    for i in range(bn_j):
                        nc.vector.bn_stats(BNS[:, i, :], X[:, t0 + i, :])
                    # sums: S[f] = 256*(m_even + m_odd)
                    be.tensor_add(bntmp[:, 0:bn_j],
                                  BNS[:, 0:bn_j, 1], BNS[:, 0:bn_j, 4])
                    be.tensor_scalar_mul(S[:, t0:t0 + bn_j],
                                         bntmp[:, 0:bn_j], 256.0)
                    # sumsqs: BNS[...,2|5] is count*var, so
                    # S[F+f] = (cv_e + cv_o) + 256*(m_e^2 + m_o^2)
                    mv = BNS[:, :, :].rearrange("p f (u c) -> p f u c", u=2)
                    be.tensor_mul(BNM[:, 0:bn_j, :],
                                  mv[:, 0:bn_j, :, 1],
                                  mv[:, 0:bn_j, :, 1])  # m^2 e|o
                    if bn_on_pool:
                        # (cv_e+cv_o) + 256*(m2_e+m2_o) via add/mult only
                        be.tensor_add(BNA[:, 0:bn_j, 0], BNM[:, 0:bn_j, 0],
                                      BNM[:, 0:bn_j, 1])
                        be.tensor_scalar_mul(BNA[:, 0:bn_j, 1],
                                             BNA[:, 0:bn_j, 0], 256.0)
                        be.tensor_add(bntmp[:, fs:fs + bn_j],
                                      mv[:, 0:bn_j, 0, 2],
                                      mv[:, 0:bn_j, 1, 2])
                        be.tensor_add(S[:, F + t0:F + t0 + bn_j],
                                      BNA[:, 0:bn_j, 1],
                                      bntmp[:, fs:fs + bn_j])
                    else:
                        nc.vector.scalar_tensor_tensor(
                            out=BNA[:, 0:bn_j, :], in0=BNM[:, 0:bn_j, :],
                            scalar=256.0, in1=mv[:, 0:bn_j, :, 2],
                            op0=mybir.AluOpType.mult,
                            op1=mybir.AluOpType.add)   # 256*m^2 + count*var
                        nc.vector.tensor_add(S[:, F + t0:F + t0 + bn_j],
                                             BNA[:, 0:bn_j, 0],
                                             BNA[:, 0:bn_j, 1])
                for i, f in enumerate(range(t0, t0 + fs)):
                    if i < bn_j and (parts & P_SQ) and (parts & P_SUMS):
                        continue
                    if not (parts & P_SQ):
                        pass
                    elif i < sq_dve_frames:
                        # sumsq on DVE: x*x tensor_tensor_reduce (2x packed)
                        nc.vector.tensor_tensor_reduce(
                            out=sqscr2[:, :], in0=X[:, f, :], in1=X[:, f, :],
                            scale=1.0, scalar=0.0,
                            op0=mybir.AluOpType.mult,
                            op1=mybir.AluOpType.add,
                            accum_out=S[:, F + f:F + f + 1],
                        )
                    else:
                        nc.scalar.activation(
                            out=sqscr[:, :], in_=X[:, f, :],
                            func=mybir.ActivationFunctionType.Square,
                            bias=zb[:, :], scale=1.0,
                            accum_out=S[:, F + f:F + f + 1],
                        )
                    if not (parts & P_SUMS):
                        pass
                    elif i < sum_act_frames:
                        # frame-sum on ACT: Identity+accum
                        nc.scalar.activation(
                            out=sumscr[:, :], in_=X[:, f, :],
                            func=mybir.ActivationFunctionType.Identity,
                            bias=zb[:, :], scale=1.0,
                            accum_out=S[:, f:f + 1],
                        )
                    elif sum_mode == "dve_ts":
                        # frame-sum on DVE: copy+accum at 4x packed mode
                        nc.vector.tensor_scalar(
                            out=sumscr[:, :], in0=X[:, f, :],
                            scalar1=1.0, scalar2=0.0,
                            op0=mybir.AluOpType.mult,
                            op1=mybir.AluOpType.add,
                            accum_out=S[:, f:f + 1],
                        )
                if (parts & P_SUMS) and sum_mode == "reduce":
                    # batched DVE reduce over the segment (1x mode)
                    nc.vector.reduce_sum(S[:, t0:t0 + fs],
                                         X[:, t0:t0 + fs, :],
                                         axis=mybir.AxisListType.X)

                if parts & P_SCAN:
                    # ---- causal scan for this segment --------------------
                    # intra-partition inclusive prefix over the FS frames
                    # each partition owns (fp32 recurrence on DVE)
                    nc.vector.tensor_tensor_scan(
                        out=C[:, t0:t0 + fs], data0=S[:, t0:t0 + fs],
                        data1=S[:, t0:t0 + fs], initial=0.0,
                        op0=mybir.AluOpType.add, op1=mybir.AluOpType.bypass)
                    nc.vector.tensor_tensor_scan(
                        out=C[:, F + t0:F + t0 + fs],
                        data0=S[:, F + t0:F + t0 + fs],
                        data1=S[:, F + t0:F + t0 + fs], initial=0.0,
                        op0=mybir.AluOpType.add, op1=mybir.AluOpType.bypass)
                    # cross-partition exclusive carry (+ prev segment total):
                    # carry[p] = sum_{q<p} seg_total[q] + prev_seg_final
                    totals = Cr[:, :, t0 + fs - 1]          # [P, 2] strided
                    nc.tensor.matmul(carryP[:, 0:2], ustrict[:, :], totals,
                                     start=True, stop=(s == 0))
                    if s > 0:
                        prevfinal = Cr[:, :, t0 - 1]        # already global
                        nc.tensor.matmul(carryP[:, 0:2], row124[:, :],
                                         prevfinal, start=False, stop=True)
                    cS = carryS[:, 2 * s:2 * s + 2]
                    nc.scalar.copy(cS[:, :], carryP[:, :])
                    nc.vector.tensor_scalar_add(C[:, t0:t0 + fs],
                                                C[:, t0:t0 + fs], cS[:, 0:1])
                    nc.vector.tensor_scalar_add(C[:, F + t0:F + t0 + fs],
                                                C[:, F + t0:F + t0 + fs],
                                                cS[:, 1:2])

                    # ---- stats for this segment --------------------------
                    # M = C * invc2: [-mean | E[x^2]] (both halves, 3D AP).
                    # C must stay intact: the next segment's carry matmul
                    # reads this segment's final cum column.
                    nc.vector.tensor_mul(Mr[:, :, t0:t0 + fs],
                                         Cr[:, :, t0:t0 + fs],
                                         Ir[:, :, t0:t0 + fs])
                    nc.vector.tensor_mul(Msq[:, t0:t0 + fs], M[:, t0:t0 + fs],
                                         M[:, t0:t0 + fs])          # mean^2
                    nc.vector.tensor_sub(V[:, t0:t0 + fs],
                                         M[:, F + t0:F + t0 + fs],
                                         Msq[:, t0:t0 + fs])        # var
                    nc.scalar.activation(out=V[:, t0:t0 + fs],
                                         in_=V[:, t0:t0 + fs],
                                         func=mybir.ActivationFunctionType.Sqrt,
                                         bias=eps_t[:, :], scale=1.0)
                    nc.vector.reciprocal(ISD[:, t0:t0 + fs], V[:, t0:t0 + fs])
                    if apply_mode == "amr" or (last_act_affines
                                               and s == nseg - 1):
                        # -mean*invstd, only where a mult-then-add form
                        # needs it (ACT affine / amr); the DVE TS affine
                        # uses add-then-mult with M directly
                        nc.vector.tensor_mul(NMB[:, t0:t0 + fs],
                                             M[:, t0:t0 + fs],
                                             ISD[:, t0:t0 + fs])

                # ---- apply + store for this segment ----------------------
                if parts & P_APPLY:
                    seg_ttmul = last_ttmul and s >= nseg - ttmul_tail
                    for f in range(t0, t0 + fs):
                        if apply_mode == "amr":
                            # out = (x*invstd + (-mean*invstd)) * gamma,
                            # fused, one rounding to fp16 at the output.
                            nc.vector.affine_mul_reduce(
                                out=X[:, f, :], accum_out=amracc[:, 0:1],
                                in0=X[:, f, :], in1=G[:, :],
                                scale=ISD[:, f:f + 1], bias=NMB[:, f:f + 1])
                        else:
                            if (s == nseg - 1 and last_act_affines
                                    and f - t0 < last_act_affines):
                                # tail: ACT is idle after its last squares,
                                # so it takes the first affines of the final
                                # segment while DVE handles rest + gammas
                                nc.scalar.activation(
                                    out=X[:, f, :], in_=X[:, f, :],
                                    func=mybir.ActivationFunctionType.Identity,
                                    bias=NMB[:, f:f + 1],
                                    scale=ISD[:, f:f + 1])
                            else:
                                # x <- (x + (-mean)) * invstd, 4x packed TS;
                                # add-then-mult uses M directly (no NMB op)
                                nc.vector.tensor_scalar(
                                    out=X[:, f, :], in0=X[:, f, :],
                                    scalar1=M[:, f:f + 1],
                                    scalar2=ISD[:, f:f + 1],
                                    op0=mybir.AluOpType.add,
                                    op1=mybir.AluOpType.mult)
                            if apply_mode == "ts_tt" or seg_ttmul:
                                # per-frame gamma: lets the final stores
                                # drain while later frames still apply
                                nc.vector.tensor_mul(X[:, f, :], X[:, f, :],
                                                     G[:, :])
                        if Bt is not None:
                            nc.vector.tensor_add(X[:, f, :], X[:, f, :],
                                                 Bt[:, :])
                    if apply_mode == "ts_ttb" and not seg_ttmul:
                        # one batched gamma multiply for the segment (2x);
                        # leading segments can offload it to the idle Pool
                        geng = (nc.gpsimd if s < pool_gamma_segs
                                else nc.vector)
                        geng.tensor_mul(X[:, t0:t0 + fs, :],
                                        X[:, t0:t0 + fs, :], G8v)
                if parts & P_STORE:
                    tail_start = nseg - (ttmul_tail if last_ttmul else 1)
                    if store_mode != "pair" or nseg < 4:
                        nc.gpsimd.dma_start(out_v[s], X[:, t0:t0 + fs, :])
                    elif s >= tail_start:
                        # tail segments apply gamma per-frame, so stores
                        # chunk finer and stream as frames complete
                        nq = 4 if s == nseg - 1 else 2
                        qq = fs // nq
                        for q in range(nq):
                            nc.gpsimd.dma_start(
                                out_v[s][:, q * qq:(q + 1) * qq],
                                X[:, t0 + q * qq:t0 + (q + 1) * qq, :])
                    elif s % 2 == 1:
                        # paired 2MB store of segments s-1, s: one SWDGE
                        # fixed cost amortized over both
                        src = X[:, t0 - fs:t0 + fs, :].rearrange(
                            "p (u f) h -> p u f h", u=2)
                        nc.gpsimd.dma_start(out_pv[:, s - 1:s + 1], src)
                    elif s == tail_start - 1 and tail_start % 2 == 1:
                        # odd leftover before the tail: single store
                        nc.gpsimd.dma_start(out_v[s], X[:, t0:t0 + fs, :])

            if hw_loop > 0:
                # hardware loop: hw_loop iterations of `reps` unrolled
                # pipelines each — total reps*hw_loop, small NEFF. Used by
                # the timing harness for a noise-robust on-device signal.
                # staggered_reset avoids the all-engine drain+barrier at the
                # back edge so iterations pipeline like unrolled reps do.
                with tc.For_i(0, hw_loop, staggered_reset=True):
                    for _rep in range(reps):
                        _rep_body()
            else:
                for _rep in range(reps):
                    if rep_barrier and _rep > 0:
                        tc.strict_bb_all_engine_barrier()
                    _rep_body()

    # Runs Bacc's compile passes (register allocation, EventSemaphore
    # legalization of multi-wait instructions, nop fusion).
    nc.finalize()
    return nc


_NC_CACHE: dict = {}


def kernel(**inputs: np.ndarray) -> np.ndarray:
    x = np.asarray(inputs["inputs"])
    gamma = np.asarray(inputs["gamma"], dtype=np.float32)
    beta = np.asarray(inputs["beta"], dtype=np.float32)
    assert x.shape == (B, K, H), x.shape

    x16 = np.ascontiguousarray(x.astype(np.float16))
    gamma16 = np.ascontiguousarray(gamma.reshape(1, H).astype(np.float16))
    beta16 = np.ascontiguousarray(beta.reshape(1, H).astype(np.float16))

    with_beta = bool(np.any(beta != 0.0))
    key = (with_beta, 1)
    if key not in _NC_CACHE:
        _NC_CACHE[key] = _build_kernel(with_beta, reps=1)
    nc = _NC_CACHE[key]

    consts = _make_consts()
    in_maps = [
        {
            "x": np.ascontiguousarray(x16[b]),
            "gamma": gamma16,
            "beta": beta16,
            "consts": consts,
        }
        for b in range(B)
    ]
    res = bass_utils.run_bass_kernel_spmd(nc, in_maps, core_ids=list(range(N_CORES)))
    out = np.stack([res.results[b]["out"] for b in range(B)], axis=0)
    return out.astype(np.float32)

